# revision 66
# baseline (speedup 1.0000x reference)
import sys

sys.path.insert(0, "/opt/trn_rl_repo")

import numpy as np
import os as _os

# Problem constants (hardcoded per harness contract)
B = 64          # full batch
NC_CORES = 8
BPC = 8         # batches per core
N = 1024
D = 768
NS = 16         # n_slots
KT = 8          # n-tiles of 128
DT = 6          # d-tiles of 128
NG = 8          # column groups per batch in blocked layout (N / 128)
BG = BPC * NG   # legacy
NGRP = 2        # phase-B groups (each 4 batches)
BPG = BPC // NGRP
PG = BPG * NG   # 32 partitions of blocked loop state per group

REPS = int(_os.environ.get("KERNEL_REPS", "1"))
# phase gating for timing attribution: "A", "AB", or "ABC" (full kernel)
PHASES = _os.environ.get("KERNEL_PHASES", "ABC")

_CACHE = {}


def _build_nc(debug=False, reps=None):
    import concourse.bacc as bacc
    import concourse.tile as tile
    import concourse.mybir as mybir

    if reps is None:
        reps = REPS

    fp32 = mybir.dt.float32
    fp16 = mybir.dt.float16

    nc = bacc.Bacc(
        "TRN2",
        target_bir_lowering=False,
        debug=False,
        enable_asserts=False,
        num_devices=NC_CORES,
    )

    f_dr = nc.dram_tensor("features", [BPC, N, D], fp32, kind="ExternalInput").ap()
    ident_dr = nc.dram_tensor("identity", [128, 128], fp32, kind="ExternalInput").ap()
    idh_dr = nc.dram_tensor("identh", [128, 128], fp16, kind="ExternalInput").ap()
    ec_dr = nc.dram_tensor("econst", [BPG, PG], fp32, kind="ExternalInput").ap()
    pc_dr = nc.dram_tensor("pcol", [NGRP, PG, 2], fp32, kind="ExternalInput").ap()
    cr_dr = nc.dram_tensor("crow", [1, 128], fp32, kind="ExternalInput").ap()
    out_dr = nc.dram_tensor("slots", [BPC, NS, D], fp32, kind="ExternalOutput").ap()
    # Raw-Gram scratch (fp32): rows (b*N + n)*NG + g of 128 cols; the greedy
    # loop gathers row (code*8 + g) per (b,g) partition and normalizes with
    # the gathered 1/||f|| scalars (rs_dr).
    g_dr = nc.dram_tensor("g_scratch", [BPC * N * NG, 128], fp32, kind="Internal").ap()
    g_wr = g_dr.rearrange("(r e) c -> r (e c)", e=NG)  # [BPC*N, N] fp32
    rs_dr = nc.dram_tensor("rsal_scratch", [BPC * N, 2], fp32, kind="Internal").ap()

    with tile.TileContext(nc) as tc:
        with (
            tc.tile_pool(name="main", bufs=1) as mp,
            tc.tile_pool(name="fbuf", bufs=2) as fbp,
            tc.tile_pool(name="sqscr", bufs=1) as sqp,
            tc.tile_pool(name="fnt", bufs=1) as ftp,
            tc.tile_pool(name="gst", bufs=2) as gsp,
            tc.tile_pool(name="small", bufs=2) as smp,
            tc.tile_pool(name="psT", bufs=2, space="PSUM") as ppT,
            tc.tile_pool(name="psG", bufs=2, space="PSUM") as ppG,
            tc.tile_pool(name="psM", bufs=2, space="PSUM") as ppM,
            tc.tile_pool(name="psB", bufs=2, space="PSUM") as ppB,
        ):
            ident = mp.tile([128, 128], fp32)
            nc.sync.dma_start(ident, ident_dr)
            identh = mp.tile([128, 128], fp16)
            nc.sync.dma_start(identh, idh_dr)
            E_sb = mp.tile([BPG, PG], fp32)
            nc.sync.dma_start(E_sb, ec_dr)
            pcol = []
            for g in range(NGRP):
                pc = mp.tile([PG, 2], fp32, name=f"pcol{g}")
                nc.sync.dma_start(pc, pc_dr[g])
                pcol.append(pc)
            crow = mp.tile([1, 128], fp32)
            nc.sync.dma_start(crow, cr_dr)

            consts = (ident, identh, E_sb, pcol, crow)
            for _rep in range(reps):
                _run_once(nc, tc, tile, mybir,
                          mp, fbp, ftp, gsp, smp, sqp, ppT, ppG, ppM, ppB,
                          consts, f_dr, out_dr, g_dr, g_wr, rs_dr)
                if reps > 1:
                    tc.strict_bb_all_engine_barrier()

    nc.compile()
    return nc


def _run_once(nc, tc, tile, mybir,
              mp, fbp, ftp, gsp, smp, sqp, ppT, ppG, ppM, ppB,
              consts, f_dr, out_dr, g_dr, g_wr, rs_dr):
    from concourse.bass import IndirectOffsetOnAxis
    from concourse.tile_rust import add_dep_helper

    fp32 = mybir.dt.float32
    fp32r = mybir.dt.float32r
    fp16 = mybir.dt.float16
    i32 = mybir.dt.int32
    u32 = mybir.dt.uint32
    u8 = mybir.dt.uint8
    Alu = mybir.AluOpType
    Act = mybir.ActivationFunctionType
    X = mybir.AxisListType.X

    ident, identh, E_sb, pcol, crow = consts

    # persistent tiles
    fnh = [mp.tile([128, KT, D], fp16, name=f"fnh{b}") for b in range(BPC)]
    pk = [mp.tile([128, BPG, KT], fp32, name=f"pk{g}") for g in range(NGRP)]
    pk2 = [mp.tile([128, BPG, KT], fp32, name=f"pk2{g}") for g in range(NGRP)]
    wT = [mp.tile([128, KT, BPG, NS], fp16, name=f"wT{g}") for g in range(NGRP)]
    wsum = [mp.tile([PG, NS], fp32, name=f"wsum{g}") for g in range(NGRP)]

    # ---------------- Phase A: per-batch RAW Gram -----------------------
    # The Gram is computed on raw features (fp32r inputs, 1 cyc/row);
    # normalization scalars 1/||f|| are applied in phase B from rmT (per
    # column) and a gathered (-r, r) pair (per selected position). Raw
    # features are kept resident in SBUF as fp16 (fnh) for phase C.
    wdeps = [[], []]   # gram-write DMA instrs per group (for gather RAW deps)
    rdeps = [[], []]   # rsal-write DMA instrs per group
    for b in range(BPC):
        grp = b // BPG
        bl = b % BPG
        sal2 = smp.tile([128, KT], fp32, tag="sal2")
        fview = f_dr[b].rearrange("(kt p) d -> p kt d", p=128)
        fT = ftp.tile([128, DT, N], fp32r, tag="fT")
        for q in range(4):
            f_sb = fbp.tile([128, 2, D], fp32, tag="f")
            nc.sync.dma_start(f_sb, fview[:, 2 * q:2 * q + 2])
            for k in range(2):
                kt = 2 * q + k
                sq = sqp.tile([128, D], fp32, tag="sq")
                nc.scalar.activation(
                    sq, f_sb[:, k], Act.Square,
                    accum_out=sal2[:, kt:kt + 1],
                )
                # raw transpose (identity): fT[d, n] = f[n, d]
                for dt0, ndt in ((0, 4), (4, 2)):
                    tp = ppT.tile([128, 512], fp32, tag="tpk")
                    for j in range(ndt):
                        nc.tensor.transpose(
                            tp[:, j * 128:(j + 1) * 128],
                            f_sb[:, k, (dt0 + j) * 128:(dt0 + j + 1) * 128],
                            ident,
                        )
                    src = tp[:, 0:ndt * 128].rearrange("p (a c) -> p a c", c=128)
                    dst = fT[:, dt0:dt0 + ndt, kt * 128:(kt + 1) * 128]
                    if (kt + dt0) % 2 == 0:
                        nc.scalar.copy(dst, src)
                    else:
                        nc.vector.tensor_copy(dst, src)
            # raw features cast to fp16, kept resident for phase C
            nc.gpsimd.tensor_copy(
                fnh[b][:, 2 * q:2 * q + 2].rearrange("p k d -> p (k d)"),
                f_sb.rearrange("p k d -> p (k d)"),
            )

        # saliency + reciprocal; (-r, r) pairs to DRAM for phase-B gathers
        nc.scalar.activation(pk[grp][:, bl], sal2, Act.Sqrt)
        nc.vector.reciprocal(pk2[grp][:, bl], pk[grp][:, bl])
        nr = smp.tile([128, KT, 2], fp32, tag="nr")
        nc.vector.tensor_scalar(
            nr[:, :, 0], pk2[grp][:, bl], -1.0, None, op0=Alu.mult
        )
        nc.vector.tensor_copy(nr[:, :, 1], pk2[grp][:, bl])
        rdma = nc.sync.dma_start(
            rs_dr[b * N:(b + 1) * N].rearrange("(kt p) c -> p kt c", p=128),
            nr,
        )
        rdeps[grp].append(rdma)

        # G_raw = f @ f.T upper triangle in 256-wide chunks; row-pairs share
        # one gstage tile so each pair is written with a single DMA
        gst_tiles = []
        for rp in range(4):
            w = N - rp * 256
            gst_tiles.append(gsp.tile([128, 2, w], fp32, tag=f"rp{rp}",
                                      name=f"rp{rp}"))
        for i in range(KT):
            rp = i // 2
            c0 = rp * 256
            for jc in range(rp, 4):
                gp = ppG.tile([128, 256], fp32, tag="gps")
                for dt in range(DT):
                    nc.tensor.matmul(
                        gp,
                        fT[:, dt, i * 128:(i + 1) * 128],
                        fT[:, dt, jc * 256:(jc + 1) * 256],
                        start=(dt == 0),
                        stop=(dt == DT - 1),
                    )
                co = jc * 256 - c0
                if (i + jc) % 2 == 0:
                    nc.scalar.copy(gst_tiles[rp][:, i % 2, co:co + 256], gp)
                else:
                    nc.vector.tensor_copy(gst_tiles[rp][:, i % 2, co:co + 256], gp)
            if i % 2 == 1:
                wdma = nc.sync.dma_start(
                    g_wr[b * N + (i - 1) * 128: b * N + (i + 1) * 128, c0:]
                    .rearrange("(two p) w -> p two w", p=128),
                    gst_tiles[rp],
                )
                wdeps[grp].append(wdma)

        # mirror blocks below the diagonal: row-block j cols [0, (j//2)*256)
        for j in range(2, KT):
            nmb = j // 2
            mst = gsp.tile([128, nmb * 256], fp32, tag="mst")
            for k in range(nmb):
                mp_ps = ppM.tile([128, 256], fp32, tag="mpk")
                so = j * 128 - k * 256
                for half in range(2):
                    nc.tensor.transpose(
                        mp_ps[:, half * 128:(half + 1) * 128],
                        gst_tiles[k][:, half, so:so + 128],
                        ident,
                    )
                if (j + k) % 2 == 0:
                    nc.scalar.copy(mst[:, k * 256:(k + 1) * 256], mp_ps)
                else:
                    nc.vector.tensor_copy(mst[:, k * 256:(k + 1) * 256], mp_ps)
            wdma = nc.sync.dma_start(
                g_wr[b * N + j * 128: b * N + (j + 1) * 128, 0:nmb * 256],
                mst,
            )
            wdeps[grp].append(wdma)

    if "B" not in PHASES:
        return

    # ---------------- Phase B: 16-step greedy loop, 2 groups ------------
    mask, msal, rmT = [], [], []
    sims, us, t1s, rsps = [], [], [], []
    mx8, ix8, vi, viT = [], [], [], []
    bmax, eq, code, nst, nT, offs, offs2 = [], [], [], [], [], [], []
    gate, w1, aggw, aggw2, simv, um, vtmp = [], [], [], [], [], [], []
    for g in range(NGRP):
        salT_ps = ppB.tile([PG, 128], fp32, tag="tps")
        nc.tensor.transpose(
            salT_ps, pk[g].rearrange("p b k -> p (b k)"), ident
        )
        ms = mp.tile([PG, 128], fp32, name=f"msal{g}")
        nc.scalar.copy(ms, salT_ps)
        msal.append(ms)
        rmT_ps = ppB.tile([PG, 128], fp32, tag="tps")
        nc.tensor.transpose(
            rmT_ps, pk2[g].rearrange("p b k -> p (b k)"), ident
        )
        rt = mp.tile([PG, 128], fp32, name=f"rmT{g}")
        nc.scalar.copy(rt, rmT_ps)
        rmT.append(rt)
        mk = mp.tile([PG, 128], fp32, name=f"mask{g}")
        nc.vector.memset(mk, 1.0)
        mask.append(mk)
        sims.append([mp.tile([PG, 128], fp32, name=f"sim{g}_{i}")
                     for i in range(2)])
        us.append([mp.tile([PG, 128], fp32, name=f"u{g}_{i}")
                   for i in range(2)])
        t1s.append([mp.tile([PG, 128], fp32, name=f"t1{g}_{i}")
                    for i in range(2)])
        rsps.append([mp.tile([PG, 2], fp32, name=f"rsp{g}_{i}")
                     for i in range(2)])
        mx8.append(mp.tile([PG, 8], fp32, name=f"mx8{g}"))
        ix8.append(mp.tile([PG, 8], u32, name=f"ix8{g}"))
        vi.append(mp.tile([PG, 1], fp32, name=f"vi{g}"))
        viT.append((mp.tile([1, PG], fp32, name=f"valT{g}"),
                    mp.tile([1, PG], fp32, name=f"idxT{g}")))
        bmax.append(mp.tile([1, BPG], fp32, name=f"bmax{g}"))
        eq.append(mp.tile([1, PG], u8, name=f"eq{g}"))
        code.append(mp.tile([1, PG], fp32, name=f"code{g}"))
        nst.append(mp.tile([1, BPG], fp32, name=f"nst{g}"))
        nT.append(mp.tile([BPG, 1], fp32, name=f"nT{g}"))
        offs.append(mp.tile([PG, 1], i32, name=f"offs{g}"))
        offs2.append(mp.tile([PG, 1], i32, name=f"offs2{g}"))
        gate.append(mp.tile([PG, 128], fp32, name=f"gate{g}"))
        w1.append(mp.tile([PG, 128], fp32, name=f"w1{g}"))
        aggw.append(mp.tile([PG, 128], fp32, name=f"aggw{g}"))
        aggw2.append(mp.tile([PG, 128], fp32, name=f"aggw2{g}"))
        simv.append(mp.tile([PG, 128], fp32, name=f"simv{g}"))
        um.append(mp.tile([PG, 128], fp32, name=f"um{g}"))
        vtmp.append(mp.tile([PG, 128], fp32, name=f"vtmp{g}"))

    def emit_deferred(g, t):
        # off-critical aggregation + mask update for step t of group g
        u = us[g][t % 2]
        # sim = (raw_row * r_m) * r_sel
        nc.vector.tensor_scalar(
            simv[g], t1s[g][t % 2], rsps[g][t % 2][:, 1:2], None, op0=Alu.mult
        )
        nc.vector.tensor_scalar(gate[g], simv[g], 0.5, None, op0=Alu.is_gt)
        nc.vector.tensor_mul(w1[g], simv[g], mask[g])
        nc.vector.tensor_mul(aggw[g], w1[g], gate[g])
        nc.scalar.activation(
            aggw2[g], aggw[g], Act.Copy, accum_out=wsum[g][:, t:t + 1]
        )
        awT_ps = ppB.tile([128, PG], fp32, tag="tps")
        nc.tensor.transpose(awT_ps, aggw[g], ident[:PG, :PG])
        nc.scalar.copy(
            wT[g][:, :, :, t],
            awT_ps.rearrange("p (b gg) -> p gg b", b=BPG),
        )
        # mask = mask * (1 - clip(sim,0,1)) = min(mask*relu(1-sim), mask)
        nc.vector.tensor_mul(um[g], mask[g], u)
        nc.vector.tensor_tensor(mask[g], um[g], mask[g], op=Alu.min)

    def emit_step(g, t):
        s = sims[g][t % 2]
        u = us[g][t % 2]
        nc.vector.max(out=mx8[g], in_=msal[g])
        nc.vector.max_index(out=ix8[g], in_max=mx8[g], in_values=msal[g])
        # local idx -> global code b*1024 + g2*128 + c
        nc.vector.tensor_scalar(
            vi[g], ix8[g][:, 0:1], pcol[g][:, 0:1], None, op0=Alu.add
        )
        valT, idxT = viT[g]
        valT_ps = ppB.tile([1, PG], fp32, tag="tps")
        nc.tensor.transpose(valT_ps, mx8[g][:, 0:1], ident[:PG, :PG])
        nc.scalar.copy(valT, valT_ps)
        idxT_ps = ppB.tile([1, PG], fp32, tag="tps")
        nc.tensor.transpose(idxT_ps, vi[g], ident[:PG, :PG])
        nc.scalar.copy(idxT, idxT_ps)
        # per-batch max over groups, first-index tiebreak via min-code
        nc.vector.tensor_reduce(
            bmax[g], valT.rearrange("o (b gg) -> o b gg", b=BPG),
            axis=X, op=Alu.max,
        )
        nc.vector.tensor_tensor(
            eq[g].rearrange("o (b gg) -> o b gg", b=BPG),
            valT.rearrange("o (b gg) -> o b gg", b=BPG),
            bmax[g].unsqueeze(2).to_broadcast([1, BPG, NG]),
            op=Alu.is_ge,
        )
        nc.vector.select(code[g], eq[g], idxT, crow[0:1, 0:PG])
        nc.vector.tensor_reduce(
            nst[g], code[g].rearrange("o (b gg) -> o b gg", b=BPG),
            axis=X, op=Alu.min,
        )
        nT_ps = ppB.tile([BPG, 1], fp32, tag="tps")
        nc.tensor.transpose(nT_ps, nst[g], ident[:1, :1])
        nc.scalar.copy(nT[g], nT_ps)
        rep_ps = ppB.tile([PG, 1], fp32, tag="tps")
        nc.tensor.matmul(rep_ps, E_sb, nT[g], start=True, stop=True)
        nc.vector.tensor_scalar(
            offs2[g], rep_ps, 1.0, None, op0=Alu.mult
        )
        nc.vector.tensor_scalar(
            offs[g], rep_ps, 8.0, pcol[g][:, 1:2], op0=Alu.mult, op1=Alu.add
        )
        rsp = rsps[g][t % 2]
        gi = nc.gpsimd.indirect_dma_start(
            out=s,
            out_offset=None,
            in_=g_dr,
            in_offset=IndirectOffsetOnAxis(ap=offs[g], axis=0),
        )
        gi2 = nc.gpsimd.indirect_dma_start(
            out=rsp,
            out_offset=None,
            in_=rs_dr,
            in_offset=IndirectOffsetOnAxis(ap=offs2[g], axis=0),
        )
        if t == 0:
            for wdma in wdeps[g]:
                add_dep_helper(gi.ins, wdma.ins, sync=True,
                               reason="gram writes visible before gather")
            for rdma in rdeps[g]:
                add_dep_helper(gi2.ins, rdma.ins, sync=True,
                               reason="rsal writes visible before gather")
        if t > 0:
            emit_deferred(g, t - 1)
        # critical tail: sim = raw*r_m*r_sel;
        # msal *= (1 - clip(sim,0,1)), via min trick
        t1 = t1s[g][t % 2]
        nc.vector.tensor_mul(t1, s, rmT[g])
        nc.scalar.activation(u, t1, Act.Relu, bias=1.0, scale=rsp[:, 0:1])
        nc.vector.tensor_mul(vtmp[g], msal[g], u)
        nc.vector.tensor_tensor(msal[g], vtmp[g], msal[g], op=Alu.min)

    for t in range(NS):
        for g in range(NGRP):
            emit_step(g, t)
    for g in range(NGRP):
        emit_deferred(g, NS - 1)

    if "C" not in PHASES:
        return

    # ---------------- Phase C: slot matmuls -------------------------
    recip = []
    for g in range(NGRP):
        wsT_ps = ppB.tile([NS, PG], fp32, tag="tps")
        nc.tensor.transpose(wsT_ps, wsum[g], ident[:PG, :PG])
        wsT = smp.tile([NS, PG], fp32, tag="wsT")
        nc.scalar.copy(wsT, wsT_ps)
        wsum_b = smp.tile([NS, BPG], fp32, tag="wsb")
        nc.vector.tensor_reduce(
            wsum_b, wsT.rearrange("p (b gg) -> p b gg", b=BPG),
            axis=X, op=Alu.add,
        )
        nc.vector.tensor_scalar(wsum_b, wsum_b, 1e-8, None, op0=Alu.add)
        rc = mp.tile([NS, BPG], fp32, name=f"recip{g}")
        nc.vector.reciprocal(rc, wsum_b)
        recip.append(rc)

    for b in range(BPC):
        g = b // BPG
        bl = b % BPG
        slot_sb = sqp.tile([NS, D], fp32, tag="slot")
        for h in range(2):
            sp = ppT.tile([NS, 384], fp32, tag="tpk")
            for kt in range(KT):
                nc.tensor.matmul(
                    sp,
                    wT[g][:, kt, bl, :],
                    fnh[b][:, kt, h * 384:(h + 1) * 384],
                    start=(kt == 0),
                    stop=(kt == KT - 1),
                )
            nc.scalar.activation(
                slot_sb[:, h * 384:(h + 1) * 384], sp, Act.Copy,
                scale=recip[g][:, bl:bl + 1]
            )
        nc.sync.dma_start(out_dr[b], slot_sb)


def _get_nc(debug=False, reps=None):
    key = ("nc", debug, reps if reps is not None else REPS, PHASES)
    if key not in _CACHE:
        _CACHE[key] = _build_nc(debug, reps=reps)
    return _CACHE[key]


def _consts():
    ident = np.eye(128, dtype=np.float32)
    identh = np.eye(128, dtype=np.float16)
    E = np.zeros((BPG, PG), dtype=np.float32)
    for p in range(PG):
        E[p // NG, p] = 1.0
    pcol = np.zeros((NGRP, PG, 2), dtype=np.float32)
    for g in range(NGRP):
        for p in range(PG):
            b = g * BPG + p // NG
            pcol[g, p, 0] = b * N + (p % NG) * 128
            pcol[g, p, 1] = p % NG
    crow = np.zeros((1, 128), dtype=np.float32)
    crow[0, :PG] = 65536.0                      # BIG (> any code)
    return ident, identh, E, pcol, crow


def _make_in_maps(feats):
    ident, identh, E, pcol, crow = _consts()
    return [
        {
            "features": feats[i * BPC:(i + 1) * BPC],
            "identity": ident,
            "identh": identh,
            "econst": E,
            "pcol": pcol,
            "crow": crow,
        }
        for i in range(NC_CORES)
    ]


def kernel(features, batch_size=None, **_kw):
    from concourse import bass_utils

    nc = _get_nc(reps=1)
    feats = np.ascontiguousarray(np.asarray(features, dtype=np.float32))
    in_maps = _make_in_maps(feats)
    res = bass_utils.run_bass_kernel_spmd(
        nc, in_maps, core_ids=list(range(NC_CORES))
    )
    outs = [np.asarray(res.results[i]["slots"]) for i in range(NC_CORES)]
    return np.concatenate(outs, axis=0).astype(np.float32)
